# revision 13
# baseline (speedup 1.0000x reference)
"""Distributed AttentionAutoEncoder kernel for 8 TRN2 NeuronCores (Bass/Tile).

Reference computation (fp32):
    Q = W_q @ X ; K = W_v @ X ; V = W_k @ X          (d=2048, n=8192)
    S = (Q @ K.T) / sqrt(d) ; Z = softmax(S, -1) ; A = Z @ V

Reformulation: S = W_q @ G @ Wvs with G = X @ X.T, Wvs = W_v.T/sqrt(d).
G contracts over n (data-parallel): each core computes its partial
G_c = X_c @ X_c.T (upper triangle + PE-transposed mirror), and a
ReduceScatter leaves 256 fully-summed G rows per core (out 2MB -- vs a
16MB AllReduce).  Each core then computes T[m_c,:] = G[m_c,:] @ Wvs and
the partial S_c = (Wq^T[m_c,:])^T-contraction with T[m_c,:]; a second
ReduceScatter gives each core 256 fully-summed S rows, softmax is fully
local, and the normalized P rows are transposed on PE and AllGathered in
fp16 (8MB, 2 chunks) for the data-parallel A = P @ V.

Precision: softmax logits are ~1e4 with near-one-hot rows, so the
G/T/S chain uses fp16 hi/lo 3-pass matmuls (hi*hi+hi*lo+lo*hi, fp32
PSUM).  V/P/A use single-pass fp16.  The two RS stages reduce in fp32.

V = W_k @ X runs in two halves that cover the two ReduceScatter windows
so PE never idles on the (serialized) collective engine.
"""

import numpy as np

import concourse.bacc as bacc
import concourse.mybir as mybir
import concourse.tile as tile
from concourse.masks import make_identity

P = 128
FP16 = mybir.dt.float16
FP32 = mybir.dt.float32
AF = mybir.ActivationFunctionType

D_FULL = 2048
N_FULL = 8192
NCORES = 8


def build(D=D_FULL, NL=N_FULL // NCORES, NC=NCORES):
    JS = D // NC          # rows of G/S owned by this core (256)
    MT = JS // P          # m-tiles per core (2)
    nT = NL // P          # n-tiles per core (8)
    dT = D // P           # d-tiles (16)
    CB = 512              # column-block width for G upper-tri store
    KB = D // CB          # 4
    NBS = 512             # n-block for V/A matmul moving dim
    NB = NL // NBS        # 2
    JB = 512              # j-block width for T/S_c moving dim
    JBN = D // JB         # 4
    grp = [list(range(NC))]

    nc = bacc.Bacc("TRN2", target_bir_lowering=False, debug=False,
                   num_devices=NC)

    # ------------- I/O -------------
    xt_hi = nc.dram_tensor("xt_hi", [NL, D], FP16, kind="ExternalInput")
    xt_lo = nc.dram_tensor("xt_lo", [NL, D], FP16, kind="ExternalInput")
    xn_hi = nc.dram_tensor("xn_hi", [D, NL], FP16, kind="ExternalInput")
    wkt_hi = nc.dram_tensor("wkt_hi", [dT // 4, dT, P, 4 * P], FP16,
                            kind="ExternalInput")
    # full Wv^T*sc, k-tile blocked, hi/lo: [k_tile, 2, P, D]
    wvs_b = nc.dram_tensor("wvs_b", [dT, 2, P, D], FP16,
                           kind="ExternalInput")
    # per-core Wq^T rows m_c, m-tile blocked, hi/lo: [MT, 2, P, D]
    wqts_b = nc.dram_tensor("wqts_b", [MT, 2, P, D], FP16,
                            kind="ExternalInput")
    a_out = nc.dram_tensor("a_out", [D, NL], FP32, kind="ExternalOutput")

    with tile.TileContext(nc) as tc:
        dpool = tc.alloc_tile_pool(name="dram", bufs=1, space="DRAM")
        g_rs_in = dpool.tile([D, D], FP32, name="g_rs_in")
        g_rows = dpool.tile([JS, D], FP32, name="g_rows")
        # S partials split by i-parity: half h holds rows {i-tile 2c+h}
        s_rs_in = [dpool.tile([D // 2, D], FP32, name=f"s_rs_in{h}")
                   for h in range(MT)]
        s_rows = [dpool.tile([P, D], FP32, name=f"s_rows{h}")
                  for h in range(MT)]
        # AG payload layout [j_in, jt, i]: SBUF-friendly on the A side
        ag_in = [dpool.tile([P, D // P, P], FP16, name=f"ag_in{h}")
                 for h in range(MT)]
        ag_out = [dpool.tile([NC, P, D // P, P], FP16, name=f"ag_out{h}",
                             addr_space="Shared") for h in range(MT)]

        id_pool = tc.alloc_tile_pool(name="ident", bufs=1)
        ident16 = id_pool.tile([P, P], FP16, name="ident16")
        make_identity(nc, ident16)
        ident32 = id_pool.tile([P, P], FP32, name="ident32")
        make_identity(nc, ident32)

        # long-lived SBUF pools; tiles allocated up-front (stack discipline)
        xn_pool = tc.alloc_tile_pool(name="xn", bufs=1)
        xn_sb = [xn_pool.tile([P, NL], FP16, name=f"xn{k}")
                 for k in range(dT)]
        wq_pool = tc.alloc_tile_pool(name="wq", bufs=1)
        v_pool = tc.alloc_tile_pool(name="vsb", bufs=1)
        v_sb = [v_pool.tile([P, NL], FP16, name=f"v{iv}")
                for iv in range(dT)]

        # =========== Phase 1: G = X X^T upper-tri + mirror ===========
        xt_pool = tc.alloc_tile_pool(name="xt", bufs=1)
        xth, xtl = [], []
        for n in range(nT):
            th = xt_pool.tile([P, D], FP16, name=f"xth{n}")
            tl = xt_pool.tile([P, D], FP16, name=f"xtl{n}")
            nc.sync.dma_start(out=th, in_=xt_hi[n * P:(n + 1) * P, :])
            nc.sync.dma_start(out=tl, in_=xt_lo[n * P:(n + 1) * P, :])
            xth.append(th)
            xtl.append(tl)
        wqts = []          # [MT][2] hi/lo [P, D]
        for mt in range(MT):
            pair = []
            for hlo in range(2):
                t = wq_pool.tile([P, D], FP16, name=f"wq{mt}_{hlo}")
                nc.sync.dma_start(out=t, in_=wqts_b[mt, hlo])
                pair.append(t)
            wqts.append(pair)

        # G upper-triangle blocks + mirror transposes.  mrow[a] stages the
        # mirrored lower strip for d-tile row a (cols 0..4*(a//4)*P).
        gstg_pool = tc.alloc_tile_pool(name="gstg", bufs=4)
        mrow_pool = tc.alloc_tile_pool(name="mrow", bufs=1)
        gps_pool = tc.alloc_tile_pool(name="gps", bufs=4, space="PSUM")
        mps_pool = tc.alloc_tile_pool(name="mps", bufs=4, space="PSUM")
        mrow = {}
        for a in range(4, dT):
            w = 4 * (a // 4) * P
            mrow[a] = mrow_pool.tile([P, w], FP32, name=f"mrow{a}")
        for m in range(dT):
            if m % 4 == 0 and m > 0:
                for a in range(m, m + 4):
                    nc.sync.dma_start(
                        out=g_rs_in[a * P:(a + 1) * P, :m * P],
                        in_=mrow[a])
            for kb in range(m // 4, KB):
                ks = slice(kb * CB, (kb + 1) * CB)
                ps = gps_pool.tile([P, CB], FP32, name="g_ps", tag="g_ps")
                acc = 0
                last = 3 * nT - 1
                for n in range(nT):
                    for lh, rh in ((xth[n], xth[n]), (xth[n], xtl[n]),
                                   (xtl[n], xth[n])):
                        nc.tensor.matmul(ps, lh[:, m * P:(m + 1) * P],
                                         rh[:, ks], start=(acc == 0),
                                         stop=(acc == last))
                        acc += 1
                stg = gstg_pool.tile([P, CB], FP32, name="g_stg",
                                     tag="g_stg")
                nc.scalar.copy(stg, ps)
                nc.sync.dma_start(out=g_rs_in[m * P:(m + 1) * P, ks],
                                  in_=stg)
                if kb > m // 4:
                    for q in range(4):
                        a = kb * 4 + q
                        mp = mps_pool.tile([P, P], FP32, name="m_ps",
                                           tag="m_ps")
                        nc.tensor.transpose(mp, stg[:, q * P:(q + 1) * P],
                                            ident32)
                        nc.vector.tensor_copy(
                            out=mrow[a][:, m * P:(m + 1) * P], in_=mp)
        for a in range(12, dT):
            nc.sync.dma_start(out=g_rs_in[a * P:(a + 1) * P, :12 * P],
                              in_=mrow[a])
        for k in range(dT):
            nc.sync.dma_start(out=xn_sb[k], in_=xn_hi[k * P:(k + 1) * P, :])
        mps_pool.release()
        gps_pool.release()
        mrow_pool.release()
        gstg_pool.release()
        xt_pool.release()

        # ---- ReduceScatter G: each core gets its m_c rows, summed ----
        nc.gpsimd.collective_compute(
            "ReduceScatter", mybir.AluOpType.add, replica_groups=grp,
            ins=[g_rs_in.opt()], outs=[g_rows.opt()])

        # =========== V (first half covers RS(G)) ===========

        def v_sweep(ivg):
            wk_pool = tc.alloc_tile_pool(name="wk", bufs=4)
            vps_pool = tc.alloc_tile_pool(name="vps", bufs=4 * NB,
                                          space="PSUM")
            pss = {}
            for j in range(4):
                for nb in range(NB):
                    pss[(j, nb)] = vps_pool.tile([P, NBS], FP32,
                                                 name="v_ps", tag="v_ps")
            for k in range(dT):
                wt = wk_pool.tile([P, 4 * P], FP16, name="wk_t", tag="wk_t")
                nc.sync.dma_start(out=wt, in_=wkt_hi[ivg // 4, k])
                for j in range(4):
                    for nb in range(NB):
                        ns = slice(nb * NBS, (nb + 1) * NBS)
                        nc.tensor.matmul(pss[(j, nb)],
                                         wt[:, j * P:(j + 1) * P],
                                         xn_sb[k][:, ns],
                                         start=(k == 0), stop=(k == dT - 1))
            for j in range(4):
                for nb in range(NB):
                    ns = slice(nb * NBS, (nb + 1) * NBS)
                    nc.vector.tensor_copy(out=v_sb[ivg + j][:, ns],
                                          in_=pss[(j, nb)])
            vps_pool.release()
            wk_pool.release()

        v_sweep(0)
        v_sweep(4)

        # =========== Phase 2: T = G[m_c,:] @ Wvs ===========
        # load + split + transpose the core's G rows
        g32_pool = tc.alloc_tile_pool(name="g32", bufs=1)
        gt_pool = tc.alloc_tile_pool(name="gt", bufs=1)
        gh, gl = [], []
        for mt in range(MT):
            g32 = g32_pool.tile([P, D], FP32, name=f"g32_{mt}")
            nc.sync.dma_start(out=g32, in_=g_rows[mt * P:(mt + 1) * P, :])
            h = g32_pool.tile([P, D], FP16, name=f"gh{mt}")
            l = g32_pool.tile([P, D], FP16, name=f"gl{mt}")
            nc.vector.tensor_copy(out=h, in_=g32)
            nc.vector.tensor_sub(l, g32, h)
            gh.append(h)
            gl.append(l)
        gth, gtl = [], []      # [k-tile][P(k), MT*P(m)] hi/lo
        for kt in range(dT):
            gth.append(gt_pool.tile([P, MT * P], FP16, name=f"gth{kt}"))
            gtl.append(gt_pool.tile([P, MT * P], FP16, name=f"gtl{kt}"))
        gtps_pool = tc.alloc_tile_pool(name="gtps", bufs=4, space="PSUM")
        for mt in range(MT):
            for kt in range(dT):
                for src, dstl in ((gh, gth), (gl, gtl)):
                    tp = gtps_pool.tile([P, P], FP16, name="gt_ps",
                                        tag="gt_ps")
                    nc.tensor.transpose(tp, src[mt][:, kt * P:(kt + 1) * P],
                                        ident16)
                    nc.vector.tensor_copy(
                        out=dstl[kt][:, mt * P:(mt + 1) * P], in_=tp)
        gtps_pool.release()

        t_pool = tc.alloc_tile_pool(name="tsb", bufs=1)
        th_t = [t_pool.tile([P, D], FP16, name=f"th{mt}")
                for mt in range(MT)]
        tl_t = [t_pool.tile([P, D], FP16, name=f"tl{mt}")
                for mt in range(MT)]
        wvs_pool = tc.alloc_tile_pool(name="wvs", bufs=4)
        tps_pool = tc.alloc_tile_pool(name="tps", bufs=2 * JBN,
                                      space="PSUM")
        pss = {}
        for mt in range(MT):
            for jb in range(JBN):
                pss[(mt, jb)] = tps_pool.tile([P, JB], FP32, name="t_ps",
                                              tag="t_ps")
        for kt in range(dT):
            wvh = wvs_pool.tile([P, D], FP16, name="wv_h", tag="wv_h")
            wvl = wvs_pool.tile([P, D], FP16, name="wv_l", tag="wv_l")
            nc.sync.dma_start(out=wvh, in_=wvs_b[kt, 0])
            nc.sync.dma_start(out=wvl, in_=wvs_b[kt, 1])
            for pi, (lh_l, rh) in enumerate(((gth, wvh), (gth, wvl),
                                             (gtl, wvh))):
                for mt in range(MT):
                    for jb in range(JBN):
                        js = slice(jb * JB, (jb + 1) * JB)
                        nc.tensor.matmul(
                            pss[(mt, jb)],
                            lh_l[kt][:, mt * P:(mt + 1) * P],
                            rh[:, js],
                            start=(kt == 0 and pi == 0),
                            stop=(kt == dT - 1 and pi == 2))
        for mt in range(MT):
            h, l = th_t[mt], tl_t[mt]
            for jb in range(JBN):
                js = slice(jb * JB, (jb + 1) * JB)
                nc.vector.tensor_copy(out=h[:, js], in_=pss[(mt, jb)])
                nc.vector.tensor_sub(l[:, js], pss[(mt, jb)], h[:, js])
        tps_pool.release()
        wvs_pool.release()

        # =========== Phase 3: partial S_c = Wq^T-slice . T ===========
        sstg_pool = tc.alloc_tile_pool(name="sstg", bufs=4)
        sps_pool = tc.alloc_tile_pool(name="sps", bufs=2 * JBN,
                                      space="PSUM")
        for h in range(MT):
            for c in range(NC):
                i = MT * c + h
                isl = slice(i * P, (i + 1) * P)
                pssi = [sps_pool.tile([P, JB], FP32, name="s_ps",
                                      tag="s_ps") for _ in range(JBN)]
                for pi, (wq_h, t_l) in enumerate(((0, th_t), (0, tl_t),
                                                  (1, th_t))):
                    for mt in range(MT):
                        for jb in range(JBN):
                            js = slice(jb * JB, (jb + 1) * JB)
                            nc.tensor.matmul(
                                pssi[jb], wqts[mt][wq_h][:, isl],
                                t_l[mt][:, js],
                                start=(pi == 0 and mt == 0),
                                stop=(pi == 2 and mt == MT - 1))
                stg = sstg_pool.tile([P, D], FP32, name="s_stg",
                                     tag="s_stg")
                for jb in range(JBN):
                    js = slice(jb * JB, (jb + 1) * JB)
                    nc.scalar.copy(stg[:, js], pssi[jb])
                nc.sync.dma_start(
                    out=s_rs_in[h][c * P:(c + 1) * P, :], in_=stg)
            # ReduceScatter this half as soon as its 8 tiles are written
            nc.gpsimd.collective_compute(
                "ReduceScatter", mybir.AluOpType.add, replica_groups=grp,
                ins=[s_rs_in[h].opt()], outs=[s_rows[h].opt()])
        sps_pool.release()
        sstg_pool.release()
        t_pool.release()
        gt_pool.release()
        g32_pool.release()

        # =========== V sweep 3 (covers RS of the odd half) ===========
        v_sweep(8)

        # =========== Phase 4: softmax + P^T + AllGather (per half) =====
        def sm_half(h):
            srow_pool = tc.alloc_tile_pool(name="srow", bufs=1)
            ptst_pool = tc.alloc_tile_pool(name="ptst", bufs=4)
            ptps_pool = tc.alloc_tile_pool(name="ptps", bufs=4,
                                           space="PSUM")
            srow = srow_pool.tile([P, D], FP32, name=f"srow{h}")
            mx = srow_pool.tile([P, 1], FP32, name=f"mx{h}")
            negm = srow_pool.tile([P, 1], FP32, name=f"negm{h}")
            ssum = srow_pool.tile([P, 1], FP32, name=f"ssum{h}")
            recip = srow_pool.tile([P, 1], FP32, name=f"recip{h}")
            nc.sync.dma_start(out=srow, in_=s_rows[h])
            nc.vector.reduce_max(mx, srow, axis=mybir.AxisListType.X)
            nc.scalar.mul(negm, mx, -1.0)
            et = srow_pool.tile([P, D], FP16, name=f"e{h}")
            nc.scalar.activation(et, srow, AF.Exp, bias=negm,
                                 scale=1.0, accum_out=ssum)
            nc.vector.reciprocal(recip, ssum)
            pt = srow_pool.tile([P, D], FP16, name=f"p{h}")
            nc.vector.tensor_scalar_mul(pt, et, recip)
            for jt in range(dT):
                tp = ptps_pool.tile([P, P], FP16, name="pt_ps", tag="pt_ps")
                nc.tensor.transpose(tp, pt[:, jt * P:(jt + 1) * P], ident16)
                stg = ptst_pool.tile([P, P], FP16, name="pt_stg",
                                     tag="pt_stg")
                nc.vector.tensor_copy(out=stg, in_=tp)
                nc.sync.dma_start(out=ag_in[h][:, jt, :], in_=stg)
            nc.gpsimd.collective_compute(
                "AllGather", mybir.AluOpType.bypass, replica_groups=grp,
                ins=[ag_in[h].opt()], outs=[ag_out[h].opt()])
            ptps_pool.release()
            ptst_pool.release()
            srow_pool.release()

        sm_half(0)
        v_sweep(12)        # fills the AG0 window
        sm_half(1)

        # =========== Phase 5: A = P @ V ===========
        pt_pool = tc.alloc_tile_pool(name="ptsb", bufs=3)
        a_pool = tc.alloc_tile_pool(name="astg", bufs=2)
        aps_pool = tc.alloc_tile_pool(name="aps", bufs=2 * NB, space="PSUM")
        for h in range(MT):
            for r in range(NC):
                t_i = r * MT + h
                ptv = pt_pool.tile([P, D], FP16, name="pt_sb", tag="pt_sb")
                src = ag_out[h][r].rearrange("j jt i -> j (jt i)")
                nc.sync.dma_start(out=ptv, in_=src)
                apss = [aps_pool.tile([P, NBS], FP32, name="a_ps",
                                      tag="a_ps") for _ in range(NB)]
                for jt in range(dT):
                    for nb in range(NB):
                        ns = slice(nb * NBS, (nb + 1) * NBS)
                        nc.tensor.matmul(apss[nb],
                                         ptv[:, jt * P:(jt + 1) * P],
                                         v_sb[jt][:, ns],
                                         start=(jt == 0),
                                         stop=(jt == dT - 1))
                ast = a_pool.tile([P, NL], FP32, name="a_stg", tag="a_stg")
                for nb in range(NB):
                    ns = slice(nb * NBS, (nb + 1) * NBS)
                    nc.scalar.copy(ast[:, ns], apss[nb])
                nc.sync.dma_start(out=a_out[t_i * P:(t_i + 1) * P, :],
                                  in_=ast)
        aps_pool.release()
        a_pool.release()
        pt_pool.release()
        v_pool.release()
        wq_pool.release()
        xn_pool.release()
        id_pool.release()
        dpool.release()

    nc.compile()
    return nc


def prepare_inputs(X_t, W_q, W_k, W_v, NC=NCORES):
    """Host-side sharding + fp16 hi/lo splits.  Returns in_maps for SPMD."""
    D, N = X_t.shape
    NL = N // NC
    JS = D // NC
    MT = JS // 128
    dT = D // 128
    P_ = 128
    sc = np.float32(1.0) / np.sqrt(np.float32(D))

    def split(a):
        hi = a.astype(np.float16)
        lo = (a.astype(np.float32) - hi.astype(np.float32)).astype(np.float16)
        return np.ascontiguousarray(hi), np.ascontiguousarray(lo)

    wkt_hi = np.ascontiguousarray(W_k.T.astype(np.float16))
    wkt_hi = np.ascontiguousarray(
        wkt_hi.reshape(dT, P_, dT // 4, 4 * P_).transpose(2, 0, 1, 3))

    wvs = (W_v.T.astype(np.float32) * sc)
    wvs_hi, wvs_lo = split(wvs)
    # [k_tile, 2, P, D]
    wvs_b = np.ascontiguousarray(
        np.stack([wvs_hi.reshape(dT, P_, D), wvs_lo.reshape(dT, P_, D)],
                 axis=1))

    wqt = np.ascontiguousarray(W_q.T.astype(np.float32))
    wqt_hi, wqt_lo = split(wqt)

    in_maps = []
    for c in range(NC):
        xc = np.ascontiguousarray(X_t[:, c * NL:(c + 1) * NL]
                                  .astype(np.float32))
        xt_hi, xt_lo = split(np.ascontiguousarray(xc.T))
        r0 = c * JS
        wq_sl_hi = wqt_hi[r0:r0 + JS].reshape(MT, P_, D)
        wq_sl_lo = wqt_lo[r0:r0 + JS].reshape(MT, P_, D)
        wqts_b = np.ascontiguousarray(
            np.stack([wq_sl_hi, wq_sl_lo], axis=1))
        in_maps.append({
            "xt_hi": xt_hi, "xt_lo": xt_lo,
            "xn_hi": np.ascontiguousarray(xc.astype(np.float16)),
            "wkt_hi": wkt_hi,
            "wvs_b": wvs_b,
            "wqts_b": wqts_b,
        })
    return in_maps


_CACHED_NC = None


def _get_nc():
    global _CACHED_NC
    if _CACHED_NC is None:
        _CACHED_NC = build()
    return _CACHED_NC


def run(X_t, W_q, W_k, W_v, trace=False):
    from concourse.bass_utils import run_bass_kernel_spmd
    nc = _get_nc()
    in_maps = prepare_inputs(X_t, W_q, W_k, W_v)
    res = run_bass_kernel_spmd(nc, in_maps, core_ids=list(range(NCORES)),
                               trace=trace)
    A = np.concatenate([res.results[c]["a_out"] for c in range(NCORES)],
                       axis=1)
    return A, res


def kernel(X_t, W_q, W_k, W_v):
    X_t = np.asarray(X_t)
    W_q = np.asarray(W_q)
    W_k = np.asarray(W_k)
    W_v = np.asarray(W_v)
    A, _ = run(X_t, W_q, W_k, W_v, trace=False)
    return A.astype(np.float32)
